# revision 49
# baseline (speedup 1.0000x reference)
"""Student-t VQ soft-assignment (ClusteringLayer) on 8 Trainium2 NeuronCores.

q[b,k] = u / sum_k u,  u = 1/(1 + |x_b - c_k|^2)   (ALPHA = 1)

Strategy (data-parallel over batch, centroid table replicated):
  host: xT = x.T cast to bf16, sharded by batch into 8x [256, 8192];
        cmat = -2 * clusters.T (bf16); csq1 = 1 + |c_k|^2 (f32, from the
        bf16-rounded clusters so it is consistent with the matmul operand);
        xsq = |x_b|^2 per row (f32, from bf16-rounded x); centered bf16
        copies of both for the PE rank-2 bias fold.
  core: per 128-row tile,
        PE  : m = -2 x.c^T via two bf16 matmuls (d split 2x128) -> PSUM;
              on ACT-path tiles a third rank-2 matmul adds
              (csq1-256)[k] + (xsq-256)[b] into PSUM.
        pass2 (split by tile):
          DVE : fused custom op r = recip1NR(m + xsq + csq1) bf16,
                accum_out = row-sum(r) f32
          ACT : r = Reciprocal(m' + 512.0) bf16, accum_out = row-sum
        DVE : sr = recip_approx_fast(row sums)
        scale (split by tile over DVE / ACT copy):
              q = r * sr  (bf16)
        DMA : q pairs -> DRAM on alternating queues (sync / gpsimd)
  host: concat + upcast bf16 -> f32.

Schedule notes (from trace analysis):
  - Only ct + xg0/xg1 DMAs are issued before the first matmul group is
    emitted; Tile coarsens DMA waits to "all DMAs outstanding at emission",
    so the other constant loads are deferred until after group-0 matmuls.
  - ~WARM dummy matmuls on a memset tile warm the PE HAM clock gate
    (cold 1.2GHz -> warm 2.4GHz needs ~3.4us of sustained PE activity)
    while the first input DMA completion semaphores post.
  - First/last groups are small so the pipeline fills fast and drains fast.
"""

import os
from contextlib import ExitStack
from operator import add as _add

import numpy as np
import ml_dtypes

N_CORES = 8
B_FULL = 65536
B_CORE = B_FULL // N_CORES  # 8192
D = 256
K = 512
TILES_TOTAL = B_CORE // 128  # 64

SIZES = [int(s) for s in os.environ.get(
    "VQ_SIZES", "2,2,2,2,4,4,6,6,6,6,6,6,6,2,2,2").split(",")]
# pass2 engine per tile: d=DVE custom op, a=ACT reciprocal (full-length map)
PAT8 = os.environ.get("VQ_PAT", "dddddd" + "ad" * 29)
# scale engine per tile: v=DVE ts_mul, s=ACT copy (12 of 64 on ACT --
# balances Scalar vs Vector busy, which bound the exec span)
SCALE8 = os.environ.get("VQ_SCALE", "vvsvvvvsvvvvvsvvvvsvvvvsvvvvsvvv")
# output DMA queue per transfer: s=sync, g=gpsimd, c=scalar
OUT_RINGS = os.environ.get("VQ_OUT_RINGS", "ssg")
QOUT = int(os.environ.get("VQ_QOUT", "2"))       # tiles per output DMA
WARM = int(os.environ.get("VQ_WARM", "8"))       # PE warm-up matmuls
NSYNC = int(os.environ.get("VQ_NSYNC", "6"))     # input groups on the sync ring
PFD = int(os.environ.get("VQ_PFD", "6"))         # input prefetch distance
# NSYNC and PFD must equal the xpool bufs (6): the gpsimd ring head is the
# (bufs+1)-th allocation, pool-backpressured until group 0 is consumed, and
# the prefetch distance then uses the full pool slack.

# 1-NR bit-flip reciprocal constants (Chebyshev pair over [-4.5,-4])
SEED_SCALE = -0.23549792
NR_CONST = 2.0017324

# fp8 e4m3 DoubleRow matmuls: one 256-deep matmul per tile.  At a warm
# clock this measured 216ns/tile (true 2x over bf16), BUT DoubleRow
# activity does not feed the PE HAM activity monitor, so the clock gate
# re-throttles to 1.2GHz mid-kernel and never recovers -- net slower than
# bf16.  Kept behind this flag for reference; default off.
FP8 = os.environ.get("VQ_FP8", "0") == "1"

LAST_EXEC_NS = None
LAST_RESULTS = None

_FUSED_OP = None
_NC_CACHE = None


def _ensure_ntff_hook():
    """This image's antenv lacks the tiny axon_hooks shim; synthesize it so
    BASS_TRACE=1 can capture an NTFF profile through libaxon_pjrt.so."""
    import sys
    import types
    try:
        import antenv.axon_hooks  # noqa: F401
        return
    except ImportError:
        pass
    try:
        import antenv
        mod = types.ModuleType("antenv.axon_hooks")
        mod._hook = None

        def set_axon_ntff_profile_hook(h):
            mod._hook = h

        def get_axon_ntff_profile_hook():
            return mod._hook

        mod.set_axon_ntff_profile_hook = set_axon_ntff_profile_hook
        mod.get_axon_ntff_profile_hook = get_axon_ntff_profile_hook
        sys.modules["antenv.axon_hooks"] = mod
        antenv.axon_hooks = mod
        from trn_agent_boot.trn_boot import _ntff_profile_via_ctypes
        set_axon_ntff_profile_hook(
            _ntff_profile_via_ctypes("/opt/axon/libaxon_pjrt.so"))
    except Exception:
        pass


def _register_fused_op():
    """Custom DVE op: out = recip1nr(in0 + s0 + in1), accum_out = row-sum(out).

    in0: PSUM m = -2 x.cT   s0: per-partition |x|^2   in1: broadcast 1+|c|^2.
    7 ALU stages + accumulator (fits the 8-slice budget).
    """
    global _FUSED_OP
    if _FUSED_OP is not None:
        return _FUSED_OP
    import concourse.dve_ops as dve_ops
    from concourse.dve_spec import (
        AluOp, Bin, C0, C1, C2, Spec, Src0, Src1, Zero, _has_src1, lower,
    )
    from concourse.dve_uop import DveOpSpec

    name = "VQ_RECIP1NR_BIAS_SUM"
    for op in dve_ops.OPS:
        if op.name == name:
            _FUSED_OP = op
            return op

    _m = (Src0 + C0) + Src1
    _n = Bin(AluOp.BITWISE_NOT, _m, _m)
    _y0 = _n * C1
    body = _y0 * (C2 - _m * _y0)

    def _ref(in0, in1, c0, c1, c2):
        m = (in0.astype(np.float32) + c0) + in1
        n = (~m.view(np.int32)).view(np.float32)
        y0 = n * c1
        y1 = y0 * (c2 - m * y0)
        return y1, y1.reshape(y1.shape[0], -1).sum(-1, keepdims=True)

    spec = Spec(body=body, accum=_add, accum_init=Zero, reference=_ref)
    row = max(dve_ops._SUB_OPCODE_FOR_NAME.values()) + 1
    shas = {}
    for ver in ("v3", "v4"):
        try:
            uops = lower(spec, ver=ver)
            shas[ver] = DveOpSpec(
                name=name, opcode=row, uops=uops, rd1_en=_has_src1(spec)
            ).sha(ver)
        except Exception:
            pass
    op = dve_ops.DveOp(name, spec, subdim=False, uops_sha=shas)
    dve_ops.OPS.append(op)
    dve_ops.CUSTOM_DVE_SPECS[name] = spec
    dve_ops._SUB_OPCODE_FOR_NAME[name] = row
    _FUSED_OP = op
    return op


def _act_recip(nc, out, in_, bias_imm, accum_out):
    """out = Reciprocal(in_ + bias_imm), accum_out = row-sum(out).

    BassScalarEngine.activation refuses Reciprocal wholesale (a guard for
    tight-tolerance kernels; the table is ~400 ULP which is far inside our
    2e-2 budget), so emit the InstActivation directly."""
    import concourse.mybir as mybir
    eng = nc.scalar
    inputs = [
        eng.lower_ap(in_),
        mybir.ImmediateValue(dtype=mybir.dt.float32, value=float(bias_imm)),
        mybir.ImmediateValue(dtype=mybir.dt.float32, value=1.0),
        mybir.ImmediateValue(dtype=mybir.dt.float32, value=0.0),
    ]
    outputs = [eng.lower_ap(out), eng.lower_ap(accum_out)]
    return eng.add_instruction(
        mybir.InstActivation(
            name=eng.bass.get_next_instruction_name(),
            func=mybir.ActivationFunctionType.Reciprocal,
            ins=inputs,
            outs=outputs,
        )
    )


def _build_nc():
    global _NC_CACHE
    key = (tuple(SIZES), PAT8, SCALE8, OUT_RINGS, QOUT, WARM, PFD, FP8)
    if _NC_CACHE is not None and _NC_CACHE[0] == key:
        return _NC_CACHE[1]
    import concourse.bass as bass
    import concourse.bacc as bacc
    import concourse.tile as tile
    import concourse.mybir as mybir

    op = _register_fused_op()
    BF = mybir.dt.bfloat16
    F8 = mybir.dt.float8e4
    XDT = F8 if FP8 else BF
    F32 = mybir.dt.float32
    ACT_COPY = mybir.ActivationFunctionType.Copy

    assert sum(SIZES) == TILES_TOTAL
    assert all(s % 2 == 0 for s in SIZES)

    nc = bacc.Bacc("TRN2", target_bir_lowering=False, debug=False,
                   num_devices=N_CORES)
    if FP8:
        # pre-interleaved on host for DoubleRow: xt[p, (t,j,b)] and
        # cm[p, (j,k)] with j = contraction plane (d = j*128 + p)
        xt = nc.dram_tensor("xt", [128, 2 * B_CORE], F8,
                            kind="ExternalInput").ap()
        cm = nc.dram_tensor("cmat", [128, 2 * K], F8,
                            kind="ExternalInput").ap()
    else:
        xt = nc.dram_tensor("xt", [D, B_CORE], BF, kind="ExternalInput").ap()
        cm = nc.dram_tensor("cmat", [D, K], BF, kind="ExternalInput").ap()
    cf = nc.dram_tensor("csxf", [128, K + TILES_TOTAL], F32,
                        kind="ExternalInput").ap()
    xr = nc.dram_tensor("xr2", [2, B_CORE + K], BF, kind="ExternalInput").ap()
    qo = nc.dram_tensor("qo", [B_CORE, K], BF, kind="ExternalOutput").ap()

    with tile.TileContext(nc) as tc, ExitStack() as ctx:
        const = ctx.enter_context(tc.tile_pool(name="const", bufs=1))
        xpool = ctx.enter_context(tc.tile_pool(name="x", bufs=6))
        rpool = ctx.enter_context(tc.tile_pool(name="r", bufs=20))
        qpool = ctx.enter_context(tc.tile_pool(name="q", bufs=8))
        spool = ctx.enter_context(tc.tile_pool(name="s", bufs=4))
        pm = ctx.enter_context(tc.tile_pool(name="pm", bufs=8, space="PSUM"))

        # --- early loads: everything the first groups need rides the SYNC
        # ring, serially, in consumption order.  The SDMA engines round-robin
        # between rings at packet granularity, so any other ring active in
        # the early window delays the first matmul's input; the gpsimd ring
        # is head-blocked by xpool backpressure (single tag, bufs=4) until
        # group 0 is consumed.  Matmul operands go first (PE is the critical
        # engine), pass-2 constants after. ---
        # cmat -> one SBUF tile [128, 1024]: for fp8 the DRAM is already
        # plane-interleaved ([p, j, k]); for bf16, cols :512 = d 0:128,
        # cols 512: = d 128:256.  The bf16 load is split in two so the very
        # first matmul depends on the smallest possible byte chain (the
        # second halves are emitted between the first tile's two matmuls).
        ctall = const.tile([128, 2 * K], XDT)
        cta = ctall[:]
        if FP8:
            nc.sync.dma_start(cta, cm)
            ct_dr = bass.AP(tensor=cta.tensor, offset=cta.offset,
                            ap=[list(cta.ap[0]), [K, 2], [1, K]])
        else:
            ct0 = ctall[:, 0:K]
            ct1 = ctall[:, K:2 * K]
            cm_lo = bass.AP(tensor=cm.tensor, offset=cm.offset,
                            ap=[[K, 128], [1, K]])
            cm_hi = bass.AP(tensor=cm.tensor, offset=cm.offset + 128 * K,
                            ap=[[K, 128], [1, K]])
            nc.sync.dma_start(ct0, cm_lo)

        g_offs = []
        off = 0
        for s in SIZES:
            g_offs.append(off)
            off += s * 128
        x_tiles = {}

        def prefetch(gi, eng):
            if gi >= len(SIZES) or gi in x_tiles:
                return
            gw = SIZES[gi] * 128
            xg = xpool.tile([128, 2 * gw], XDT, tag="xg")
            xga = xg[:]
            if FP8:
                # host layout is already tile/plane-interleaved: plain copy
                src = bass.AP(tensor=xt.tensor,
                              offset=xt.offset + 2 * g_offs[gi],
                              ap=[[2 * B_CORE, 128], [1, 2 * gw]])
                eng.dma_start(xga, src)
            else:
                # dram [2, 128, gw] -> sbuf [128, 2*gw] (xa = xg[:, :gw] is
                # d-rows 0:128, xb = xg[:, gw:] is 128:256)
                src = bass.AP(tensor=xt.tensor, offset=xt.offset + g_offs[gi],
                              ap=[[B_CORE, 128], [128 * B_CORE, 2], [1, gw]])
                dst = bass.AP(tensor=xga.tensor, offset=xga.offset,
                              ap=[list(xga.ap[0]), [gw, 2], [1, gw]])
                eng.dma_start(dst, src)
            x_tiles[gi] = xg

        def load_x_cols(gi, b0, bn, eng):
            """Load b-columns [b0, b0+bn) of group gi (both d-halves)."""
            gw = SIZES[gi] * 128
            xga = x_tiles[gi][:]
            src = bass.AP(tensor=xt.tensor,
                          offset=xt.offset + g_offs[gi] + b0,
                          ap=[[B_CORE, 128], [128 * B_CORE, 2], [1, bn]])
            dst = bass.AP(tensor=xga.tensor, offset=xga.offset + b0,
                          ap=[list(xga.ap[0]), [gw, 2], [1, bn]])
            eng.dma_start(dst, src)

        if FP8:
            prefetch(0, nc.sync)
        else:
            # group 0 split: only tile 0's columns before the first matmul,
            # on the SCALAR ring so they transfer in parallel with ct0 on
            # sync (the first matmul waits on both rings' first DMA only)
            xg0 = xpool.tile([128, 2 * SIZES[0] * 128], XDT, tag="xg")
            x_tiles[0] = xg0
            load_x_cols(0, 0, 128, nc.scalar)

        # pass-2 constants ride the SCALAR ring behind group 0's x halves:
        # matmuls only wait on DMAs of queues they depend on, counted at
        # emission time, so these are emitted inside the loop after the
        # first tile's matmuls (see below).
        csxf = const.tile([128, K + TILES_TOTAL], F32)
        csqb = csxf[:, 0:K]
        xsf = csxf[:, K:K + TILES_TOTAL]
        xr2 = const.tile([2, B_CORE + K], BF)
        xs2 = xr2[:, 0:B_CORE]
        rh2 = xr2[:, B_CORE:B_CORE + K]

        def load_pass2_consts():
            nc.scalar.dma_start(csxf[:], cf[:, :])
            nc.scalar.dma_start(xr2[:], xr[:, :])

        # --- PE warm-up: the HAM clock gate needs ~3.4us of PE activity to
        # lift the 1.2GHz cold throttle; run dummy matmuls on a memset tile
        # while the first input DMAs complete. ---
        wsrc = const.tile([128, K], BF)
        nc.vector.memset(wsrc[:], 0.5)
        wps = pm.tile([128, K], F32, tag="pmm")
        for _ in range(WARM):
            nc.tensor.matmul(wps[:], wsrc[:, 0:128], wsrc[:],
                             start=True, stop=True)

        out_n = [0]
        n_out_total = sum((s + QOUT - 1) // QOUT for s in SIZES)

        def out_ring():
            # the gpsimd (SWDGE) ring pays a multi-us drain after its final
            # DMA; keep the last transfers off it so that drain overlaps
            # compute instead of extending the tail.  The final two DMAs go
            # to scalar+sync so they drain in parallel (the ACT engine is
            # done computing by then).
            if out_n[0] >= n_out_total - 2:
                ring = "c" if out_n[0] == n_out_total - 2 else "s"
            else:
                ring = OUT_RINGS[out_n[0] % len(OUT_RINGS)]
            out_n[0] += 1
            return {"s": nc.sync, "g": nc.gpsimd, "c": nc.scalar}[ring]

        def emit_scales(pend):
            """Scale + out-DMA for a finished group (runs one group behind
            pass-2, so the in-order engine queues never stall on row-sums)."""
            p_gc, p_t, p_size, p_sg, p_srg, p_rt = pend
            i0 = 0
            while i0 < p_size:
                nq = min(QOUT, p_size - i0)
                q = qpool.tile([128, QOUT * K], BF, tag="q")
                for h in range(nq):
                    i = i0 + h
                    ti = p_t + i
                    sc = SCALE8[ti % len(SCALE8)]
                    if sc == "s":
                        nc.scalar.activation(q[:, h * K:(h + 1) * K],
                                             p_rt[i][:], ACT_COPY,
                                             bias=0.0, scale=p_srg[:, i:i + 1])
                    else:
                        nc.vector.tensor_scalar_mul(q[:, h * K:(h + 1) * K],
                                                    p_rt[i][:],
                                                    p_srg[:, i:i + 1])
                # nq*128 contiguous output rows -> one DMA
                row = p_gc + i0 * 128
                qa = q[:]
                src_q = bass.AP(tensor=qa.tensor, offset=qa.offset,
                                ap=[list(qa.ap[0]), [K, nq], [1, K]])
                dst_q = bass.AP(tensor=qo.tensor, offset=qo.offset + row * K,
                                ap=[[K, 128], [128 * K, nq], [1, K]])
                out_ring().dma_start(dst_q, src_q)
                i0 += nq

        pending = None
        gc = 0
        t = 0
        for gi, size in enumerate(SIZES):
            gw = size * 128
            xg = x_tiles[gi]

            s_g = spool.tile([128, size], F32, tag="s")
            sr_g = spool.tile([128, size], F32, tag="sr")

            r_tiles = []
            for i in range(size):
                c0 = i * 128
                ti = t + i
                pmm = pm.tile([128, K], F32)
                is_act = PAT8[ti % len(PAT8)] == "a"
                if FP8:
                    xga = xg[:]
                    lhsT = bass.AP(tensor=xga.tensor,
                                   offset=xga.offset + i * 256,
                                   ap=[list(xga.ap[0]), [128, 2], [1, 128]])
                    nc.tensor.matmul(pmm[:], lhsT, ct_dr,
                                     start=True, stop=not is_act,
                                     perf_mode=mybir.MatmulPerfMode.DoubleRow,
                                     skip_group_check=True)
                else:
                    nc.tensor.matmul(pmm[:], xg[:, c0:c0 + 128], ct0,
                                     start=True, stop=False)
                    if gi == 0 and i == 0:
                        # second halves of the split early loads: emitted
                        # between the first tile's matmuls so only the
                        # second matmul (and later tiles) wait on them
                        nc.sync.dma_start(ct1, cm_hi)
                        load_x_cols(0, 128, gw - 128, nc.scalar)
                    nc.tensor.matmul(pmm[:], xg[:, gw + c0:gw + c0 + 128],
                                     ct1, start=False, stop=not is_act)
                if gi == 0 and i == 0:
                    load_pass2_consts()
                r = rpool.tile([128, K], BF, tag="r")
                if is_act:
                    nc.tensor.matmul(pmm[:], xs2[:, ti * 128:(ti + 1) * 128],
                                     rh2, start=False, stop=True,
                                     skip_group_check=FP8)
                    _act_recip(nc, out=r[:], in_=pmm[:], bias_imm=512.0,
                               accum_out=s_g[:, i:i + 1])
                else:
                    nc.vector._custom_dve(
                        op, out=r[:], in0=pmm[:], in1=csqb[:],
                        s0=xsf[:, ti:ti + 1], s1=SEED_SCALE, imm2=NR_CONST,
                        accum_out=s_g[:, i:i + 1],
                    )
                r_tiles.append(r)

            # prefetch BEFORE the previous group's output DMAs so the input
            # lands ahead of them in the ring FIFO (outputs have slack;
            # a starved PE stalls everything)
            if gi == 0:
                for g2 in range(1, NSYNC):
                    prefetch(g2, nc.sync)
            prefetch(gi + PFD, nc.gpsimd)
            if pending is not None:
                emit_scales(pending)
            if gi == len(SIZES) - 1 and size == 2:
                # tail: per-tile sr so tile t's scale isn't gated on tile
                # t+1's recip
                nc.vector.reciprocal_approx_fast(out=sr_g[:, 0:1],
                                                 in_=s_g[:, 0:1])
                nc.vector.reciprocal_approx_fast(out=sr_g[:, 1:2],
                                                 in_=s_g[:, 1:2])
            else:
                nc.vector.reciprocal_approx_fast(out=sr_g[:], in_=s_g[:])
            pending = (gc, t, size, s_g, sr_g, r_tiles)
            gc += gw
            t += size
        emit_scales(pending)

    nc.compile()
    _NC_CACHE = (key, nc)
    return nc


def kernel(x, clusters):
    """Full inputs in, full output out. Shards over 8 NeuronCores inside."""
    global LAST_EXEC_NS, LAST_RESULTS
    if os.environ.get("BASS_TRACE"):
        _ensure_ntff_hook()
    from concourse.bass_utils import run_bass_kernel_spmd

    x = np.asarray(x, dtype=np.float32)
    clusters = np.asarray(clusters, dtype=np.float32)

    if FP8:
        # quantize the matmul operands to e4m3; keep |x|^2, |c|^2 exact from
        # the ORIGINAL f32 values (measured lower error than consistent-sq:
        # only the cross term carries quantization error)
        x8 = x.astype(ml_dtypes.float8_e4m3)
        c8 = clusters.astype(ml_dtypes.float8_e4m3)
        c832 = c8.astype(np.float32)
        # -2*e4m3 value is exactly representable (exponent bump)
        cmat8 = (c832.T * -2.0).astype(ml_dtypes.float8_e4m3)  # [256, 512]
        # plane-interleave for DoubleRow: cm[p, j, k], d = j*128 + p
        cmat = np.ascontiguousarray(
            np.stack([cmat8[:128], cmat8[128:]], axis=1).reshape(128, 2 * K))
        x8t = np.ascontiguousarray(x8.T)                       # [256, 65536]
        csq1 = (1.0 + (clusters.astype(np.float64) ** 2).sum(1)).astype(
            np.float32)
        xsq = (x.astype(np.float64) ** 2).sum(1).astype(np.float32)
    else:
        xbf = x.astype(ml_dtypes.bfloat16)
        xbf32 = xbf.astype(np.float32)
        xt = np.ascontiguousarray(xbf32.T).astype(ml_dtypes.bfloat16)
        cb = clusters.astype(ml_dtypes.bfloat16)
        cbf = cb.astype(np.float32)
        cmat = np.ascontiguousarray(cbf.T * -2.0).astype(ml_dtypes.bfloat16)
        csq1 = (1.0 + (cbf.astype(np.float64) ** 2).sum(1)).astype(np.float32)
        xsq = (xbf32.astype(np.float64) ** 2).sum(1).astype(np.float32)
    csq1b = np.broadcast_to(csq1[None, :], (128, K))               # [128, 512]

    # rank-2 bias fold operands (centered so bf16 abs error stays small)
    rh2 = np.stack([csq1 - 256.0,
                    np.ones(K, np.float32)]).astype(ml_dtypes.bfloat16)

    nc = _build_nc()
    in_maps = []
    for c in range(N_CORES):
        lo, hi = c * B_CORE, (c + 1) * B_CORE
        if FP8:
            # tile/plane-interleave: shard[p, (t, j, b)], d = j*128 + p
            sh = x8t[:, lo:hi]
            a = sh[:128].reshape(128, TILES_TOTAL, 128)
            b = sh[128:].reshape(128, TILES_TOTAL, 128)
            shard = np.ascontiguousarray(
                np.stack([a, b], axis=2).reshape(128, 2 * B_CORE))
        else:
            shard = np.ascontiguousarray(xt[:, lo:hi])
        xsq_c = xsq[lo:hi]
        xsq2 = np.stack([np.ones(B_CORE, np.float32),
                         xsq_c - 256.0]).astype(ml_dtypes.bfloat16)
        xr2 = np.concatenate([xsq2, rh2], axis=1)
        xsqf = np.ascontiguousarray(xsq_c.reshape(TILES_TOTAL, 128).T)
        csxf = np.ascontiguousarray(
            np.concatenate([csq1b, xsqf], axis=1))  # [128, 576] f32
        in_maps.append({"xt": shard, "cmat": cmat, "csxf": csxf,
                        "xr2": xr2})

    res = run_bass_kernel_spmd(nc, in_maps, core_ids=list(range(N_CORES)))
    LAST_RESULTS = res
    LAST_EXEC_NS = res.exec_time_ns
    out = np.concatenate([res.results[c]["qo"] for c in range(N_CORES)],
                         axis=0).astype(np.float32)
    return out


if __name__ == "__main__":
    rng = np.random.default_rng(0)
    x = rng.standard_normal((B_FULL, D), dtype=np.float32)
    c = rng.standard_normal((K, D), dtype=np.float32)
    q = kernel(x, c)
    print("out", q.shape, q.dtype, "row0 sum", q[0].sum())


# revision 50
# speedup vs baseline: 1.0143x; 1.0143x over previous
"""Student-t VQ soft-assignment (ClusteringLayer) on 8 Trainium2 NeuronCores.

q[b,k] = u / sum_k u,  u = 1/(1 + |x_b - c_k|^2)   (ALPHA = 1)

Strategy (data-parallel over batch, centroid table replicated):
  host: xT = x.T cast to bf16, sharded by batch into 8x [256, 8192];
        cmat = -2 * clusters.T (bf16); csq1 = 1 + |c_k|^2 (f32, from the
        bf16-rounded clusters so it is consistent with the matmul operand);
        xsq = |x_b|^2 per row (f32, from bf16-rounded x); centered bf16
        copies of both for the PE rank-2 bias fold.
  core: per 128-row tile,
        PE  : m = -2 x.c^T via two bf16 matmuls (d split 2x128) -> PSUM;
              on ACT-path tiles a third rank-2 matmul adds
              (csq1-256)[k] + (xsq-256)[b] into PSUM.
        pass2 (split by tile):
          DVE : fused custom op r = recip1NR(m + xsq + csq1) bf16,
                accum_out = row-sum(r) f32
          ACT : r = Reciprocal(m' + 512.0) bf16, accum_out = row-sum
        DVE : sr = recip_approx_fast(row sums)
        scale (split by tile over DVE / ACT copy):
              q = r * sr  (bf16)
        DMA : q pairs -> DRAM on alternating queues (sync / gpsimd)
  host: concat + upcast bf16 -> f32.

Schedule notes (from trace analysis):
  - Only ct + xg0/xg1 DMAs are issued before the first matmul group is
    emitted; Tile coarsens DMA waits to "all DMAs outstanding at emission",
    so the other constant loads are deferred until after group-0 matmuls.
  - ~WARM dummy matmuls on a memset tile warm the PE HAM clock gate
    (cold 1.2GHz -> warm 2.4GHz needs ~3.4us of sustained PE activity)
    while the first input DMA completion semaphores post.
  - First/last groups are small so the pipeline fills fast and drains fast.
"""

import os
from contextlib import ExitStack
from operator import add as _add

import numpy as np
import ml_dtypes

N_CORES = 8
B_FULL = 65536
B_CORE = B_FULL // N_CORES  # 8192
D = 256
K = 512
TILES_TOTAL = B_CORE // 128  # 64

SIZES = [int(s) for s in os.environ.get(
    "VQ_SIZES", "2,2,2,2,4,4,6,6,6,6,6,6,6,2,2,2").split(",")]
# pass2 engine per tile: d=DVE custom op, a=ACT reciprocal (full-length map)
PAT8 = os.environ.get("VQ_PAT", "dddddd" + "ad" * 29)
# scale engine per tile: v=DVE ts_mul, s=ACT copy (12 of 64 on ACT --
# balances Scalar vs Vector busy, which bound the exec span)
SCALE8 = os.environ.get("VQ_SCALE", "vvsvvvvsvvvvvsvvvvsvvvvsvvvvsvvv")
# output DMA queue per transfer: s=sync, g=gpsimd, c=scalar
OUT_RINGS = os.environ.get("VQ_OUT_RINGS", "ssg")
QOUT = int(os.environ.get("VQ_QOUT", "2"))       # tiles per output DMA
WARM = int(os.environ.get("VQ_WARM", "8"))       # PE warm-up matmuls
NSYNC = int(os.environ.get("VQ_NSYNC", "6"))     # input groups on the sync ring
PFD = int(os.environ.get("VQ_PFD", "6"))         # input prefetch distance
# NSYNC and PFD must equal the xpool bufs (6): the gpsimd ring head is the
# (bufs+1)-th allocation, pool-backpressured until group 0 is consumed, and
# the prefetch distance then uses the full pool slack.

# 1-NR bit-flip reciprocal constants (Chebyshev pair over [-4.5,-4])
SEED_SCALE = -0.23549792
NR_CONST = 2.0017324

# fp8 e4m3 DoubleRow matmuls: one 256-deep matmul per tile.  At a warm
# clock this measured 216ns/tile (true 2x over bf16), BUT DoubleRow
# activity does not feed the PE HAM activity monitor, so the clock gate
# re-throttles to 1.2GHz mid-kernel and never recovers -- net slower than
# bf16.  Kept behind this flag for reference; default off.
FP8 = os.environ.get("VQ_FP8", "0") == "1"

LAST_EXEC_NS = None
LAST_RESULTS = None

_FUSED_OP = None
_NC_CACHE = None


def _ensure_ntff_hook():
    """This image's antenv lacks the tiny axon_hooks shim; synthesize it so
    BASS_TRACE=1 can capture an NTFF profile through libaxon_pjrt.so."""
    import sys
    import types
    try:
        import antenv.axon_hooks  # noqa: F401
        return
    except ImportError:
        pass
    try:
        import antenv
        mod = types.ModuleType("antenv.axon_hooks")
        mod._hook = None

        def set_axon_ntff_profile_hook(h):
            mod._hook = h

        def get_axon_ntff_profile_hook():
            return mod._hook

        mod.set_axon_ntff_profile_hook = set_axon_ntff_profile_hook
        mod.get_axon_ntff_profile_hook = get_axon_ntff_profile_hook
        sys.modules["antenv.axon_hooks"] = mod
        antenv.axon_hooks = mod
        from trn_agent_boot.trn_boot import _ntff_profile_via_ctypes
        set_axon_ntff_profile_hook(
            _ntff_profile_via_ctypes("/opt/axon/libaxon_pjrt.so"))
    except Exception:
        pass


def _register_fused_op():
    """Custom DVE op: out = recip1nr(in0 + s0 + in1), accum_out = row-sum(out).

    in0: PSUM m = -2 x.cT   s0: per-partition |x|^2   in1: broadcast 1+|c|^2.
    7 ALU stages + accumulator (fits the 8-slice budget).
    """
    global _FUSED_OP
    if _FUSED_OP is not None:
        return _FUSED_OP
    import concourse.dve_ops as dve_ops
    from concourse.dve_spec import (
        AluOp, Bin, C0, C1, C2, Spec, Src0, Src1, Zero, _has_src1, lower,
    )
    from concourse.dve_uop import DveOpSpec

    name = "VQ_RECIP1NR_BIAS_SUM"
    for op in dve_ops.OPS:
        if op.name == name:
            _FUSED_OP = op
            return op

    _m = (Src0 + C0) + Src1
    _n = Bin(AluOp.BITWISE_NOT, _m, _m)
    _y0 = _n * C1
    body = _y0 * (C2 - _m * _y0)

    def _ref(in0, in1, c0, c1, c2):
        m = (in0.astype(np.float32) + c0) + in1
        n = (~m.view(np.int32)).view(np.float32)
        y0 = n * c1
        y1 = y0 * (c2 - m * y0)
        return y1, y1.reshape(y1.shape[0], -1).sum(-1, keepdims=True)

    spec = Spec(body=body, accum=_add, accum_init=Zero, reference=_ref)
    row = max(dve_ops._SUB_OPCODE_FOR_NAME.values()) + 1
    shas = {}
    for ver in ("v3", "v4"):
        try:
            uops = lower(spec, ver=ver)
            shas[ver] = DveOpSpec(
                name=name, opcode=row, uops=uops, rd1_en=_has_src1(spec)
            ).sha(ver)
        except Exception:
            pass
    op = dve_ops.DveOp(name, spec, subdim=False, uops_sha=shas)
    dve_ops.OPS.append(op)
    dve_ops.CUSTOM_DVE_SPECS[name] = spec
    dve_ops._SUB_OPCODE_FOR_NAME[name] = row
    _FUSED_OP = op
    return op


def _act_recip(nc, out, in_, bias_imm, accum_out):
    """out = Reciprocal(in_ + bias_imm), accum_out = row-sum(out).

    BassScalarEngine.activation refuses Reciprocal wholesale (a guard for
    tight-tolerance kernels; the table is ~400 ULP which is far inside our
    2e-2 budget), so emit the InstActivation directly."""
    import concourse.mybir as mybir
    eng = nc.scalar
    inputs = [
        eng.lower_ap(in_),
        mybir.ImmediateValue(dtype=mybir.dt.float32, value=float(bias_imm)),
        mybir.ImmediateValue(dtype=mybir.dt.float32, value=1.0),
        mybir.ImmediateValue(dtype=mybir.dt.float32, value=0.0),
    ]
    outputs = [eng.lower_ap(out), eng.lower_ap(accum_out)]
    return eng.add_instruction(
        mybir.InstActivation(
            name=eng.bass.get_next_instruction_name(),
            func=mybir.ActivationFunctionType.Reciprocal,
            ins=inputs,
            outs=outputs,
        )
    )


def _build_nc():
    global _NC_CACHE
    key = (tuple(SIZES), PAT8, SCALE8, OUT_RINGS, QOUT, WARM, PFD, FP8)
    if _NC_CACHE is not None and _NC_CACHE[0] == key:
        return _NC_CACHE[1]
    import concourse.bass as bass
    import concourse.bacc as bacc
    import concourse.tile as tile
    import concourse.mybir as mybir

    op = _register_fused_op()
    BF = mybir.dt.bfloat16
    F8 = mybir.dt.float8e4
    XDT = F8 if FP8 else BF
    F32 = mybir.dt.float32
    ACT_COPY = mybir.ActivationFunctionType.Copy

    assert sum(SIZES) == TILES_TOTAL
    assert all(s % 2 == 0 for s in SIZES)

    nc = bacc.Bacc("TRN2", target_bir_lowering=False, debug=False,
                   num_devices=N_CORES)
    if FP8:
        # pre-interleaved on host for DoubleRow: xt[p, (t,j,b)] and
        # cm[p, (j,k)] with j = contraction plane (d = j*128 + p)
        xt = nc.dram_tensor("xt", [128, 2 * B_CORE], F8,
                            kind="ExternalInput").ap()
        cm = nc.dram_tensor("cmat", [128, 2 * K], F8,
                            kind="ExternalInput").ap()
    else:
        xt = nc.dram_tensor("xt", [D, B_CORE], BF, kind="ExternalInput").ap()
        cm = nc.dram_tensor("cmat", [D, K], BF, kind="ExternalInput").ap()
    cf = nc.dram_tensor("csxf", [128, K + TILES_TOTAL], F32,
                        kind="ExternalInput").ap()
    xr = nc.dram_tensor("xr2", [2, B_CORE + K], BF, kind="ExternalInput").ap()
    qo = nc.dram_tensor("qo", [B_CORE, K], BF, kind="ExternalOutput").ap()

    with tile.TileContext(nc) as tc, ExitStack() as ctx:
        const = ctx.enter_context(tc.tile_pool(name="const", bufs=1))
        xpool = ctx.enter_context(tc.tile_pool(name="x", bufs=6))
        rpool = ctx.enter_context(tc.tile_pool(name="r", bufs=20))
        qpool = ctx.enter_context(tc.tile_pool(name="q", bufs=6))
        spool = ctx.enter_context(tc.tile_pool(name="s", bufs=4))
        pm = ctx.enter_context(tc.tile_pool(name="pm", bufs=8, space="PSUM"))

        # --- early loads: everything the first groups need rides the SYNC
        # ring, serially, in consumption order.  The SDMA engines round-robin
        # between rings at packet granularity, so any other ring active in
        # the early window delays the first matmul's input; the gpsimd ring
        # is head-blocked by xpool backpressure (single tag, bufs=4) until
        # group 0 is consumed.  Matmul operands go first (PE is the critical
        # engine), pass-2 constants after. ---
        # cmat -> one SBUF tile [128, 1024]: for fp8 the DRAM is already
        # plane-interleaved ([p, j, k]); for bf16, cols :512 = d 0:128,
        # cols 512: = d 128:256.  The bf16 load is split in two so the very
        # first matmul depends on the smallest possible byte chain (the
        # second halves are emitted between the first tile's two matmuls).
        ctall = const.tile([128, 2 * K], XDT)
        cta = ctall[:]
        if FP8:
            nc.sync.dma_start(cta, cm)
            ct_dr = bass.AP(tensor=cta.tensor, offset=cta.offset,
                            ap=[list(cta.ap[0]), [K, 2], [1, K]])
        else:
            ct0 = ctall[:, 0:K]
            ct1 = ctall[:, K:2 * K]
            cm_lo = bass.AP(tensor=cm.tensor, offset=cm.offset,
                            ap=[[K, 128], [1, K]])
            cm_hi = bass.AP(tensor=cm.tensor, offset=cm.offset + 128 * K,
                            ap=[[K, 128], [1, K]])
            nc.sync.dma_start(ct0, cm_lo)

        g_offs = []
        off = 0
        for s in SIZES:
            g_offs.append(off)
            off += s * 128
        x_tiles = {}

        def prefetch(gi, eng):
            if gi >= len(SIZES) or gi in x_tiles:
                return
            gw = SIZES[gi] * 128
            xg = xpool.tile([128, 2 * gw], XDT, tag="xg")
            xga = xg[:]
            if FP8:
                # host layout is already tile/plane-interleaved: plain copy
                src = bass.AP(tensor=xt.tensor,
                              offset=xt.offset + 2 * g_offs[gi],
                              ap=[[2 * B_CORE, 128], [1, 2 * gw]])
                eng.dma_start(xga, src)
            else:
                # dram [2, 128, gw] -> sbuf [128, 2*gw] (xa = xg[:, :gw] is
                # d-rows 0:128, xb = xg[:, gw:] is 128:256)
                src = bass.AP(tensor=xt.tensor, offset=xt.offset + g_offs[gi],
                              ap=[[B_CORE, 128], [128 * B_CORE, 2], [1, gw]])
                dst = bass.AP(tensor=xga.tensor, offset=xga.offset,
                              ap=[list(xga.ap[0]), [gw, 2], [1, gw]])
                eng.dma_start(dst, src)
            x_tiles[gi] = xg

        def load_x_cols(gi, b0, bn, eng):
            """Load b-columns [b0, b0+bn) of group gi (both d-halves)."""
            gw = SIZES[gi] * 128
            xga = x_tiles[gi][:]
            src = bass.AP(tensor=xt.tensor,
                          offset=xt.offset + g_offs[gi] + b0,
                          ap=[[B_CORE, 128], [128 * B_CORE, 2], [1, bn]])
            dst = bass.AP(tensor=xga.tensor, offset=xga.offset + b0,
                          ap=[list(xga.ap[0]), [gw, 2], [1, bn]])
            eng.dma_start(dst, src)

        if FP8:
            prefetch(0, nc.sync)
        else:
            # group 0 split: only tile 0's columns before the first matmul,
            # on the SCALAR ring so they transfer in parallel with ct0 on
            # sync (the first matmul waits on both rings' first DMA only)
            xg0 = xpool.tile([128, 2 * SIZES[0] * 128], XDT, tag="xg")
            x_tiles[0] = xg0
            load_x_cols(0, 0, 128, nc.scalar)

        # pass-2 constants ride the SCALAR ring behind group 0's x halves:
        # matmuls only wait on DMAs of queues they depend on, counted at
        # emission time, so these are emitted inside the loop after the
        # first tile's matmuls (see below).
        csxf = const.tile([128, K + TILES_TOTAL], F32)
        csqb = csxf[:, 0:K]
        xsf = csxf[:, K:K + TILES_TOTAL]
        xr2 = const.tile([2, B_CORE + K], BF)
        xs2 = xr2[:, 0:B_CORE]
        rh2 = xr2[:, B_CORE:B_CORE + K]

        def load_pass2_consts():
            nc.scalar.dma_start(csxf[:], cf[:, :])
            nc.scalar.dma_start(xr2[:], xr[:, :])

        # --- PE warm-up: the HAM clock gate needs ~3.4us of PE activity to
        # lift the 1.2GHz cold throttle; run dummy matmuls on a memset tile
        # while the first input DMAs complete. ---
        wsrc = const.tile([128, K], BF)
        nc.vector.memset(wsrc[:], 0.5)
        wps = pm.tile([128, K], F32, tag="pmm")
        for _ in range(WARM):
            nc.tensor.matmul(wps[:], wsrc[:, 0:128], wsrc[:],
                             start=True, stop=True)

        out_n = [0]
        n_out_total = sum((s + QOUT - 1) // QOUT for s in SIZES)

        def out_ring():
            # the gpsimd (SWDGE) ring pays a multi-us drain after its final
            # DMA; keep the last transfers off it so that drain overlaps
            # compute instead of extending the tail.  The final two DMAs go
            # to scalar+sync so they drain in parallel (the ACT engine is
            # done computing by then).
            if out_n[0] >= n_out_total - 2:
                ring = "c" if out_n[0] == n_out_total - 2 else "s"
            else:
                ring = OUT_RINGS[out_n[0] % len(OUT_RINGS)]
            out_n[0] += 1
            return {"s": nc.sync, "g": nc.gpsimd, "c": nc.scalar}[ring]

        def emit_scales(pend):
            """Scale + out-DMA for a finished group (runs one group behind
            pass-2, so the in-order engine queues never stall on row-sums)."""
            p_gc, p_t, p_size, p_sg, p_srg, p_rt = pend
            i0 = 0
            while i0 < p_size:
                nq = min(QOUT, p_size - i0)
                q = qpool.tile([128, QOUT * K], BF, tag="q")
                for h in range(nq):
                    i = i0 + h
                    ti = p_t + i
                    sc = SCALE8[ti % len(SCALE8)]
                    if sc == "s":
                        nc.scalar.activation(q[:, h * K:(h + 1) * K],
                                             p_rt[i][:], ACT_COPY,
                                             bias=0.0, scale=p_srg[:, i:i + 1])
                    else:
                        nc.vector.tensor_scalar_mul(q[:, h * K:(h + 1) * K],
                                                    p_rt[i][:],
                                                    p_srg[:, i:i + 1])
                # nq*128 contiguous output rows -> one DMA
                row = p_gc + i0 * 128
                qa = q[:]
                src_q = bass.AP(tensor=qa.tensor, offset=qa.offset,
                                ap=[list(qa.ap[0]), [K, nq], [1, K]])
                dst_q = bass.AP(tensor=qo.tensor, offset=qo.offset + row * K,
                                ap=[[K, 128], [128 * K, nq], [1, K]])
                out_ring().dma_start(dst_q, src_q)
                i0 += nq

        pending = None
        gc = 0
        t = 0
        for gi, size in enumerate(SIZES):
            gw = size * 128
            xg = x_tiles[gi]

            s_g = spool.tile([128, size], F32, tag="s")
            sr_g = spool.tile([128, size], F32, tag="sr")

            r_tiles = []
            for i in range(size):
                c0 = i * 128
                ti = t + i
                pmm = pm.tile([128, K], F32)
                is_act = PAT8[ti % len(PAT8)] == "a"
                if FP8:
                    xga = xg[:]
                    lhsT = bass.AP(tensor=xga.tensor,
                                   offset=xga.offset + i * 256,
                                   ap=[list(xga.ap[0]), [128, 2], [1, 128]])
                    nc.tensor.matmul(pmm[:], lhsT, ct_dr,
                                     start=True, stop=not is_act,
                                     perf_mode=mybir.MatmulPerfMode.DoubleRow,
                                     skip_group_check=True)
                else:
                    nc.tensor.matmul(pmm[:], xg[:, c0:c0 + 128], ct0,
                                     start=True, stop=False)
                    if gi == 0 and i == 0:
                        # second halves of the split early loads: emitted
                        # between the first tile's matmuls so only the
                        # second matmul (and later tiles) wait on them
                        nc.sync.dma_start(ct1, cm_hi)
                        load_x_cols(0, 128, gw - 128, nc.scalar)
                    nc.tensor.matmul(pmm[:], xg[:, gw + c0:gw + c0 + 128],
                                     ct1, start=False, stop=not is_act)
                if gi == 0 and i == 0:
                    load_pass2_consts()
                r = rpool.tile([128, K], BF, tag="r")
                if is_act:
                    nc.tensor.matmul(pmm[:], xs2[:, ti * 128:(ti + 1) * 128],
                                     rh2, start=False, stop=True,
                                     skip_group_check=FP8)
                    _act_recip(nc, out=r[:], in_=pmm[:], bias_imm=512.0,
                               accum_out=s_g[:, i:i + 1])
                else:
                    nc.vector._custom_dve(
                        op, out=r[:], in0=pmm[:], in1=csqb[:],
                        s0=xsf[:, ti:ti + 1], s1=SEED_SCALE, imm2=NR_CONST,
                        accum_out=s_g[:, i:i + 1],
                    )
                r_tiles.append(r)

            # prefetch BEFORE the previous group's output DMAs so the input
            # lands ahead of them in the ring FIFO (outputs have slack;
            # a starved PE stalls everything)
            if gi == 0:
                for g2 in range(1, NSYNC):
                    prefetch(g2, nc.sync)
            prefetch(gi + PFD, nc.gpsimd)
            if pending is not None:
                emit_scales(pending)
            if gi == len(SIZES) - 1 and size == 2:
                # tail: per-tile sr so tile t's scale isn't gated on tile
                # t+1's recip
                nc.vector.reciprocal_approx_fast(out=sr_g[:, 0:1],
                                                 in_=s_g[:, 0:1])
                nc.vector.reciprocal_approx_fast(out=sr_g[:, 1:2],
                                                 in_=s_g[:, 1:2])
            else:
                nc.vector.reciprocal_approx_fast(out=sr_g[:], in_=s_g[:])
            pending = (gc, t, size, s_g, sr_g, r_tiles)
            gc += gw
            t += size
        emit_scales(pending)

    nc.compile()
    _NC_CACHE = (key, nc)
    return nc


def kernel(x, clusters):
    """Full inputs in, full output out. Shards over 8 NeuronCores inside."""
    global LAST_EXEC_NS, LAST_RESULTS
    if os.environ.get("BASS_TRACE"):
        _ensure_ntff_hook()
    from concourse.bass_utils import run_bass_kernel_spmd

    x = np.asarray(x, dtype=np.float32)
    clusters = np.asarray(clusters, dtype=np.float32)

    if FP8:
        # quantize the matmul operands to e4m3; keep |x|^2, |c|^2 exact from
        # the ORIGINAL f32 values (measured lower error than consistent-sq:
        # only the cross term carries quantization error)
        x8 = x.astype(ml_dtypes.float8_e4m3)
        c8 = clusters.astype(ml_dtypes.float8_e4m3)
        c832 = c8.astype(np.float32)
        # -2*e4m3 value is exactly representable (exponent bump)
        cmat8 = (c832.T * -2.0).astype(ml_dtypes.float8_e4m3)  # [256, 512]
        # plane-interleave for DoubleRow: cm[p, j, k], d = j*128 + p
        cmat = np.ascontiguousarray(
            np.stack([cmat8[:128], cmat8[128:]], axis=1).reshape(128, 2 * K))
        x8t = np.ascontiguousarray(x8.T)                       # [256, 65536]
        csq1 = (1.0 + (clusters.astype(np.float64) ** 2).sum(1)).astype(
            np.float32)
        xsq = (x.astype(np.float64) ** 2).sum(1).astype(np.float32)
    else:
        xbf = x.astype(ml_dtypes.bfloat16)
        xbf32 = xbf.astype(np.float32)
        xt = np.ascontiguousarray(xbf32.T).astype(ml_dtypes.bfloat16)
        cb = clusters.astype(ml_dtypes.bfloat16)
        cbf = cb.astype(np.float32)
        cmat = np.ascontiguousarray(cbf.T * -2.0).astype(ml_dtypes.bfloat16)
        csq1 = (1.0 + (cbf.astype(np.float64) ** 2).sum(1)).astype(np.float32)
        xsq = (xbf32.astype(np.float64) ** 2).sum(1).astype(np.float32)
    csq1b = np.broadcast_to(csq1[None, :], (128, K))               # [128, 512]

    # rank-2 bias fold operands (centered so bf16 abs error stays small)
    rh2 = np.stack([csq1 - 256.0,
                    np.ones(K, np.float32)]).astype(ml_dtypes.bfloat16)

    nc = _build_nc()
    in_maps = []
    for c in range(N_CORES):
        lo, hi = c * B_CORE, (c + 1) * B_CORE
        if FP8:
            # tile/plane-interleave: shard[p, (t, j, b)], d = j*128 + p
            sh = x8t[:, lo:hi]
            a = sh[:128].reshape(128, TILES_TOTAL, 128)
            b = sh[128:].reshape(128, TILES_TOTAL, 128)
            shard = np.ascontiguousarray(
                np.stack([a, b], axis=2).reshape(128, 2 * B_CORE))
        else:
            shard = np.ascontiguousarray(xt[:, lo:hi])
        xsq_c = xsq[lo:hi]
        xsq2 = np.stack([np.ones(B_CORE, np.float32),
                         xsq_c - 256.0]).astype(ml_dtypes.bfloat16)
        xr2 = np.concatenate([xsq2, rh2], axis=1)
        xsqf = np.ascontiguousarray(xsq_c.reshape(TILES_TOTAL, 128).T)
        csxf = np.ascontiguousarray(
            np.concatenate([csq1b, xsqf], axis=1))  # [128, 576] f32
        in_maps.append({"xt": shard, "cmat": cmat, "csxf": csxf,
                        "xr2": xr2})

    res = run_bass_kernel_spmd(nc, in_maps, core_ids=list(range(N_CORES)))
    LAST_RESULTS = res
    LAST_EXEC_NS = res.exec_time_ns
    out = np.concatenate([res.results[c]["qo"] for c in range(N_CORES)],
                         axis=0).astype(np.float32)
    return out


if __name__ == "__main__":
    rng = np.random.default_rng(0)
    x = rng.standard_normal((B_FULL, D), dtype=np.float32)
    c = rng.standard_normal((K, D), dtype=np.float32)
    q = kernel(x, c)
    print("out", q.shape, q.dtype, "row0 sum", q[0].sum())
